# revision 1
# baseline (speedup 1.0000x reference)
"""MAB (multihead attention block) Trainium2 kernel.

Sharding: 8 cores = 4 batches x 2 query-halves. Each core computes, for its
batch b and query half s (1024 queries), the full 8-head attention block:
    q = Q @ Wq.T + bq ; k = V @ Wk.T + bk ; v = V @ Wv.T   (bv folded out)
    S = q k^T / sqrt(512); masked softmax over keys; O = q + A @ v + bv
    out = O + relu(O @ Wo.T + bo)

On-chip layouts (per core):
  qT, kT  feature-major [512, Nq/Nk]  (heads = 64-row blocks)
  v       token-major   [Nk, 772]     (per-head blocks with a mask column so
                                       the numerator matmul also accumulates
                                       the masked softmax denominator)
  logits  computed as S^T [keys, queries] so exp+mask+denominator work in
          the key-on-partition layout; 2 heads packed into the PE array via
          row tiling (K=64 each).
All matmuls run as float32r (full PE rate at free-dim >= 256).
"""

import math
import os

import numpy as np

import concourse.bass as bass
import concourse.tile as tile
from concourse import bacc, mybir

F32 = mybir.dt.float32
MM_DT = mybir.dt.float32r  # matmul operand dtype (bitcast view of f32)

DIM = 512
NQ = 1024  # queries per core
NK = 2048  # keys per core
H = 8
D = 64
P = 128
FCH = DIM // P  # 4 feature chunks
KD = DIM // P  # 4 contraction chunks
TCH = NK // P  # 16 token/key chunks
QCH = NQ // 512  # 2 query chunks of 512
SCALE = 1.0 / math.sqrt(DIM)

# v_sb per-token-chunk column layout: 4 even-head blocks of 65 (v[64] | mask),
# then 4 odd-head blocks of 128 (mask | zeros[63] | v[64]).
VW = 4 * 65 + 4 * 128  # 772
EVEN_OFF = [65 * i for i in range(4)]
ODD_OFF = [260 + 128 * i for i in range(4)]

INPUT_SPECS = {
    "QT": (DIM, NQ),
    "VT": (DIM, NK),
    "WqT": (DIM, DIM),
    "WkT": (DIM, DIM),
    "WvTp": (DIM, VW),
    "WoT": (DIM, DIM),
    "bq": (DIM,),
    "bk": (DIM,),
    "bv": (DIM,),
    "bo": (DIM,),
    "mask01": (NK,),
    "maskrep": (NK, 4),
}


def _r(ap):
    return ap.bitcast(MM_DT)


def emit(ctx, tc, io):
    """Emit the kernel. io: dict name -> DRAM AP (inputs + 'outT')."""
    nc = tc.nc
    AF = mybir.ActivationFunctionType
    OP = mybir.AluOpType

    consts = ctx.enter_context(tc.tile_pool(name="consts", bufs=1))
    bigs = ctx.enter_context(tc.tile_pool(name="bigs", bufs=1))

    # ---- constants -------------------------------------------------------
    bq_sb = consts.tile([P, FCH], F32)
    nc.sync.dma_start(bq_sb, io["bq"].rearrange("(c p) -> p c", p=P))
    bk_sb = consts.tile([P, FCH], F32)
    nc.sync.dma_start(bk_sb, io["bk"].rearrange("(c p) -> p c", p=P))
    bv_sb = consts.tile([P, FCH], F32)
    nc.sync.dma_start(bv_sb, io["bv"].rearrange("(c p) -> p c", p=P))
    bo_sb = consts.tile([P, FCH], F32)
    nc.sync.dma_start(bo_sb, io["bo"].rearrange("(c p) -> p c", p=P))
    mask_sb = consts.tile([P, TCH], F32)
    nc.sync.dma_start(mask_sb, io["mask01"].rearrange("(c p) -> p c", p=P))
    mrep_sb = consts.tile([P, TCH, 4], F32)
    nc.sync.dma_start(mrep_sb, io["maskrep"].rearrange("(c p) r -> p c r", p=P))

    # warm the ACT exp table early so the ~2.7us table load overlaps DMA
    warm = consts.tile([1, 1], F32)
    nc.vector.memset(warm, 0.0)
    nc.scalar.activation(warm, warm, AF.Exp)

    # ---- weights / inputs ------------------------------------------------
    wvp_sb = consts.tile([P, KD, VW], MM_DT)
    nc.sync.dma_start(wvp_sb, io["WvTp"].bitcast(MM_DT).rearrange("(kd p) f -> p kd f", p=P))
    wq_sb = consts.tile([P, KD, DIM], MM_DT)
    nc.sync.dma_start(wq_sb, io["WqT"].bitcast(MM_DT).rearrange("(kd p) f -> p kd f", p=P))
    wk_sb = consts.tile([P, KD, DIM], MM_DT)
    nc.sync.dma_start(wk_sb, io["WkT"].bitcast(MM_DT).rearrange("(kd p) f -> p kd f", p=P))
    wo_sb = consts.tile([P, KD, DIM], MM_DT)
    nc.sync.dma_start(wo_sb, io["WoT"].bitcast(MM_DT).rearrange("(kd p) f -> p kd f", p=P))

    vt_src = io["VT"].bitcast(MM_DT).rearrange("(kd p) t -> p kd t", p=P)
    qt_src = io["QT"].bitcast(MM_DT).rearrange("(kd p) t -> p kd t", p=P)
    with tc.tile_pool(name="ins", bufs=1) as ins_pool:
        vtin = ins_pool.tile([P, KD, NK], MM_DT)
        for kd in range(KD):
            nc.sync.dma_start(vtin[:, kd, :], vt_src[:, kd, :])
        qtin = ins_pool.tile([P, KD, NQ], MM_DT)
        for kd in range(KD):
            nc.sync.dma_start(qtin[:, kd, :], qt_src[:, kd, :])

        # ---- persistent results ------------------------------------------
        v_sb = bigs.tile([P, TCH, VW], MM_DT)
        qt_sb = bigs.tile([P, FCH, NQ], MM_DT)
        kt_sb = bigs.tile([P, FCH, NK], MM_DT)
        ot_sb = bigs.tile([P, FCH, NQ], MM_DT)

        # ---- v projection (token-major, permuted heads + mask cols) ------
        with tc.tile_pool(name="ps_v", bufs=2, space="PSUM") as ps_v_pool:
            for t in range(TCH):
                ps_v = ps_v_pool.tile([P, VW], F32, tag="psv")
                for kd in range(KD):
                    lhsT = vtin[:, kd, t * P:(t + 1) * P]
                    nc.tensor.matmul(
                        ps_v[:, 0:512], lhsT, wvp_sb[:, kd, 0:512],
                        start=(kd == 0), stop=(kd == KD - 1),
                    )
                    nc.tensor.matmul(
                        ps_v[:, 512:VW], lhsT, wvp_sb[:, kd, 512:VW],
                        start=(kd == 0), stop=(kd == KD - 1),
                    )
                # zero masked tokens (rows); mask cols are 0 here
                nc.vector.tensor_scalar_mul(v_sb[:, t, :], ps_v, mask_sb[:, t:t + 1])
                # write the mask value into the per-head mask columns
                even_cols = v_sb[:, t, 0:260].rearrange("p (e c) -> p e c", c=65)[:, :, 64]
                nc.vector.tensor_copy(even_cols, mrep_sb[:, t, :])
                odd_cols = v_sb[:, t, 260:VW].rearrange("p (o c) -> p o c", c=128)[:, :, 0]
                nc.vector.tensor_copy(odd_cols, mrep_sb[:, t, :])

        # ---- qT / kT projections (feature-major) -------------------------
        with tc.tile_pool(name="ps_p", bufs=4, space="PSUM") as ps_p_pool:
            for fc in range(FCH):
                for qc in range(QCH):
                    ps = ps_p_pool.tile([P, 512], F32, tag="psp")
                    for kd in range(KD):
                        nc.tensor.matmul(
                            ps,
                            wq_sb[:, kd, fc * P:(fc + 1) * P],
                            qtin[:, kd, qc * 512:(qc + 1) * 512],
                            start=(kd == 0), stop=(kd == KD - 1),
                        )
                    nc.vector.tensor_scalar_add(
                        qt_sb[:, fc, qc * 512:(qc + 1) * 512], ps, bq_sb[:, fc:fc + 1]
                    )
            for fc in range(FCH):
                for n in range(NK // 512):
                    ps = ps_p_pool.tile([P, 512], F32, tag="psp")
                    for kd in range(KD):
                        nc.tensor.matmul(
                            ps,
                            wk_sb[:, kd, fc * P:(fc + 1) * P],
                            vtin[:, kd, n * 512:(n + 1) * 512],
                            start=(kd == 0), stop=(kd == KD - 1),
                        )
                    nc.vector.tensor_scalar_add(
                        kt_sb[:, fc, n * 512:(n + 1) * 512], ps, bk_sb[:, fc:fc + 1]
                    )

    # ---- attention -------------------------------------------------------
    att = ctx.enter_context(tc.tile_pool(name="att", bufs=5))
    sm = ctx.enter_context(tc.tile_pool(name="sm", bufs=3))
    dr = ctx.enter_context(tc.tile_pool(name="dr", bufs=2, space="DRAM"))
    from contextlib import ExitStack as _ES

    attps = _ES()
    ps_s_pool = attps.enter_context(tc.tile_pool(name="ps_s", bufs=2, space="PSUM"))
    ps_n_pool = attps.enter_context(tc.tile_pool(name="ps_n", bufs=4, space="PSUM"))

    for pr in range(FCH):  # head pair (2pr, 2pr+1)
        for qc in range(QCH):
            num0 = ps_n_pool.tile([65, 512], F32, tag="num")
            num1 = ps_n_pool.tile([P, 512], F32, tag="num")
            for kc in range(TCH):
                s_ps = ps_s_pool.tile([P, 1024], F32, tag="s")
                for hh in range(2):
                    nc.tensor.matmul(
                        s_ps[:, hh * 512:(hh + 1) * 512],
                        kt_sb[64 * hh:64 * hh + 64, pr, kc * P:(kc + 1) * P],
                        qt_sb[64 * hh:64 * hh + 64, pr, qc * 512:(qc + 1) * 512],
                        start=True, stop=True,
                        tile_position=(64 * hh, 0),
                    )
                es = att.tile([P, 1024], MM_DT, tag="es")
                nc.scalar.activation(es, s_ps, AF.Exp, scale=SCALE)
                nc.tensor.matmul(
                    num0,
                    v_sb[:, kc, EVEN_OFF[pr]:EVEN_OFF[pr] + 65],
                    es[:, 0:512],
                    start=(kc == 0), stop=(kc == TCH - 1),
                )
                nc.tensor.matmul(
                    num1,
                    v_sb[:, kc, ODD_OFF[pr]:ODD_OFF[pr] + 128],
                    es[:, 512:1024],
                    start=(kc == 0), stop=(kc == TCH - 1),
                )
            for hh in range(2):
                num = num0 if hh == 0 else num1
                drow = num[64:65, :] if hh == 0 else num[0:1, :]
                rec = sm.tile([65, 512], F32, tag="rec")
                rslice = rec[64:65, :] if hh == 0 else rec[0:1, :]
                nc.vector.reciprocal(rslice, drow)
                drec = dr.tile([1, 512], F32, tag="drec")
                nc.sync.dma_start(drec, rslice)
                bca = sm.tile([P, 512], F32, tag="bca")
                bsl = slice(64 * hh, 64 * hh + 64)
                nc.sync.dma_start(bca[bsl, :], drec.to_broadcast([64, 512]))
                t1 = sm.tile([P, 512], F32, tag="t1")
                nsl = slice(0, 64) if hh == 0 else slice(64, 128)
                nc.vector.tensor_tensor(t1[bsl, :], num[nsl, :], bca[bsl, :], op=OP.mult)
                nc.vector.scalar_tensor_tensor(
                    ot_sb[bsl, pr, qc * 512:(qc + 1) * 512],
                    t1[bsl, :],
                    bv_sb[bsl, pr:pr + 1],
                    qt_sb[bsl, pr, qc * 512:(qc + 1) * 512].bitcast(F32),
                    op0=OP.add, op1=OP.add,
                )

    attps.close()

    # ---- output projection ----------------------------------------------
    out_dst = io["outT"].rearrange("(fc p) q -> p fc q", p=P)
    with tc.tile_pool(name="ps_u", bufs=2, space="PSUM") as ps_u_pool:
        for ofc in range(FCH):
            for qc in range(QCH):
                ups = ps_u_pool.tile([P, 512], F32, tag="psu")
                for ifc in range(FCH):
                    nc.tensor.matmul(
                        ups,
                        wo_sb[:, ifc, ofc * P:(ofc + 1) * P],
                        ot_sb[:, ifc, qc * 512:(qc + 1) * 512],
                        start=(ifc == 0), stop=(ifc == FCH - 1),
                    )
                r1 = sm.tile([P, 512], F32, tag="r1")
                nc.vector.tensor_scalar(
                    r1, ups, bo_sb[:, ofc:ofc + 1], 0.0, op0=OP.add, op1=OP.max
                )
                fin = sm.tile([P, 512], F32, tag="fin")
                nc.vector.tensor_tensor(
                    fin, r1, ot_sb[:, ofc, qc * 512:(qc + 1) * 512].bitcast(F32), op=OP.add
                )
                nc.sync.dma_start(out_dst[:, ofc, qc * 512:(qc + 1) * 512], fin)


def make_core_inputs(Q, V, mask, Wq, bq, Wk, bk, Wv, bv, Wo, bo, core):
    b, s = divmod(core, 2)
    f32 = np.float32
    QT = np.ascontiguousarray(Q[b, s * NQ:(s + 1) * NQ, :].T, dtype=f32)
    VT = np.ascontiguousarray(V[b].T, dtype=f32)
    WvT = np.ascontiguousarray(Wv.T, dtype=f32)
    WvTp = np.zeros((DIM, VW), dtype=f32)
    for i in range(4):  # even heads 2i
        WvTp[:, EVEN_OFF[i]:EVEN_OFF[i] + 64] = WvT[:, (2 * i) * 64:(2 * i + 1) * 64]
    for i in range(4):  # odd heads 2i+1
        WvTp[:, ODD_OFF[i] + 64:ODD_OFF[i] + 128] = WvT[:, (2 * i + 1) * 64:(2 * i + 2) * 64]
    m01 = mask[b].astype(f32)
    return {
        "QT": QT,
        "VT": VT,
        "WqT": np.ascontiguousarray(Wq.T, dtype=f32),
        "WkT": np.ascontiguousarray(Wk.T, dtype=f32),
        "WvTp": WvTp,
        "WoT": np.ascontiguousarray(Wo.T, dtype=f32),
        "bq": np.asarray(bq, dtype=f32),
        "bk": np.asarray(bk, dtype=f32),
        "bv": np.asarray(bv, dtype=f32),
        "bo": np.asarray(bo, dtype=f32),
        "mask01": m01,
        "maskrep": np.ascontiguousarray(np.repeat(m01[:, None], 4, axis=1)),
    }


_CACHE = {}


def build_program():
    if "nc" in _CACHE:
        return _CACHE["nc"]
    from contextlib import ExitStack

    nc = bacc.Bacc("TRN2", target_bir_lowering=False, debug=False)
    io = {}
    for name, shape in INPUT_SPECS.items():
        io[name] = nc.dram_tensor(name, list(shape), F32, kind="ExternalInput").ap()
    io["outT"] = nc.dram_tensor("outT", [DIM, NQ], F32, kind="ExternalOutput").ap()
    with tile.TileContext(nc) as tc:
        with ExitStack() as ctx:
            emit(ctx, tc, io)
    nc.compile()
    _CACHE["nc"] = nc
    return nc


def kernel(Q, V, mask, Wq, bq, Wk, bk, Wv, bv, Wo, bo):
    from concourse.bass_utils import run_bass_kernel_spmd

    nc = build_program()
    args = (Q, V, mask, Wq, bq, Wk, bk, Wv, bv, Wo, bo)
    in_maps = [make_core_inputs(*args, core=c) for c in range(8)]
    res = run_bass_kernel_spmd(
        nc, in_maps, core_ids=list(range(8)),
        trace=bool(int(os.environ.get("KTRACE", "0"))),
    )
    _CACHE["last_result"] = res
    B = 4
    out = np.empty((B, 2 * NQ, DIM), np.float32)
    for c in range(8):
        b, s = divmod(c, 2)
        out[b, s * NQ:(s + 1) * NQ, :] = res.results[c]["outT"].T
    return out



# revision 26
# speedup vs baseline: 2.7484x; 2.7484x over previous
"""MAB (multihead attention block) Trainium2 kernel, v2.

Sharding: 8 cores = 4 batches x 2 query-halves. Each core computes, for its
batch b and query half s (1024 queries), the full 8-head attention block:
    q = Q @ Wq.T + bq ; k = V @ Wk.T + bk ; v = V @ Wv.T + bv
    S = q k^T / sqrt(512); masked softmax over keys; O = q + A @ v
    out = O + relu(O @ Wo.T + bo)

Key algebraic folds (all exact up to fp rounding):
  - bk is dropped: softmax over keys is invariant to the per-query shift
    q·bk, and masked keys are excluded exactly anyway.
  - the key mask enters as a per-key exp bias (-30 => es ~ 5e-14 ~ 0)
    instead of zeroing v rows.
  - bq enters the logits as the per-key shift bq·k = (bq @ Wk) @ V — one
    extra column in the v-projection weights, added to the exp bias; the
    residual gets bq via bvq = bv + bq in the final fused add.
  - the host scales Wk/Wv by WS=64 (keeps fp8 in normal range); WS cancels:
    exp scale becomes SCALE/WS, and the softmax denominator column is WS so
    num/den is unscaled.
  With that, every PSUM evict is a pure dtype-converting copy.

Engine/dtype plan:
  - k/v projections: fp8e4 DoubleRow matmuls (contraction 2x256) with
    host-prepped [K,2,*] layouts; q/O projections and S/A@v: bf16.
  - attention per head over all 1024 queries: S^T [128 keys, 1024 q] in
    PSUM -> exp on ACT (the binding engine, ~1us per key-chunk) -> numerator
    matmul with a WS column appended to v so the masked-softmax denominator
    accumulates in the same PSUM tile.
  - normalize: reciprocal_approx_fast (DVE) + partition_broadcast (GPSIMD).
  - PSUM: two pools x 2 bufs x 2 banks = 8 banks, uniform [*, 1024] tiles.
  - DMAs chunked/ordered so the first exp fires ~12us in; all evictions run
    on DVE which trickles behind the ACT exp stream.
"""

import math
import os

import numpy as np

import concourse.bass as bass
import concourse.tile as tile
from concourse import bacc, mybir

F32 = mybir.dt.float32
BF16 = mybir.dt.bfloat16
F8 = mybir.dt.float8e4

DIM = 512
NQ = 1024  # queries per core
NK = 2048  # keys per core
H = 8
D = 64
P = 128
FCH = DIM // P  # 4 feature chunks
KD = DIM // P  # 4 contraction chunks (bf16 path)
TCH = NK // P  # 16 token/key chunks
SCALE = 1.0 / math.sqrt(DIM)
WS = 64.0  # host-side fp8 weight scale; cancels via exp-scale + den column
MB = -30.0  # masked-key exp bias: exp(s*SCALE + MB) ~ 5e-14

# v-projection output columns: 8 head blocks of [v(64) | den-slot | pad],
# then one bq·k column + pad to a 16-multiple (DoubleRow step constraint).
HB = 66
VW = H * HB  # 528: per head [v(64) | den-slot(WS) | bq_h·k_h col]
VWP = VW

INPUT_SPECS = {
    "VTd": ([P, 2, 2, NK], F8),
    "QTbf": ([P, KD, NQ], BF16),
    "WVPd": ([P, 2, 2, VWP], F8),
    "WKd": ([P, 2, 2, DIM], F8),
    "WQbf": ([P, KD, DIM], BF16),
    "WObf": ([P, KD, DIM], BF16),
    "bvq": ([DIM], F32),
    "bo": ([DIM], F32),
    "bneg": ([P, TCH], F32),
}


def emit(ctx, tc, io):
    """Emit the kernel. io: dict name -> DRAM AP (inputs + 'outT')."""
    nc = tc.nc
    AF = mybir.ActivationFunctionType
    OP = mybir.AluOpType
    DR = mybir.MatmulPerfMode.DoubleRow

    consts = ctx.enter_context(tc.tile_pool(name="consts", bufs=1))
    bigs = ctx.enter_context(tc.tile_pool(name="bigs", bufs=1))

    # ---- DMA issue order: small consts, v-proj deps, attention-h0 critical
    # path (wk, wq, qtin), then the rest ----------------------------------
    bvq_sb = consts.tile([P, FCH], F32)
    nc.sync.dma_start(bvq_sb, io["bvq"].rearrange("(c p) -> p c", p=P))
    bo_sb = consts.tile([P, FCH], F32)
    nc.sync.dma_start(bo_sb, io["bo"].rearrange("(c p) -> p c", p=P))
    bneg_sb = consts.tile([P, TCH], F32)
    nc.sync.dma_start(bneg_sb, io["bneg"])

    # warm the ACT exp table early so the ~2.7us table load overlaps DMA
    warm = consts.tile([1, 1], F32)
    nc.vector.memset(warm, 0.0)
    nc.scalar.activation(warm, warm, AF.Exp)

    wvp_d = consts.tile([P, 2, 2, VWP], F8)
    nc.sync.dma_start(wvp_d, io["WVPd"])
    vt_d = consts.tile([P, 2, 2, NK], F8)
    for tch in range(4):
        sl = slice(tch * 512, (tch + 1) * 512)
        nc.sync.dma_start(vt_d[:, :, :, sl], io["VTd"][:, :, :, sl])
    wq_sb = consts.tile([P, KD, DIM], BF16)
    nc.sync.dma_start(wq_sb, io["WQbf"])
    qtin = consts.tile([P, KD, NQ], BF16)
    for qch in range(2):
        sl = slice(qch * 512, (qch + 1) * 512)
        nc.sync.dma_start(qtin[:, :, sl], io["QTbf"][:, :, sl])
    wk_d = consts.tile([P, 2, 2, DIM], F8)
    nc.sync.dma_start(wk_d, io["WKd"])
    wo_sb = consts.tile([P, KD, DIM], BF16)
    nc.sync.dma_start(wo_sb, io["WObf"])

    # ---- persistent results ----------------------------------------------
    v_sb = bigs.tile([P, TCH, VW], BF16)
    kt_sb = bigs.tile([P, FCH, NK], BF16)
    qt_sb = bigs.tile([P, FCH, NQ], BF16)
    ot_sb = bigs.tile([P, FCH, NQ], BF16)
    ebias_sb = bigs.tile([P, TCH, H], F32)

    # softmax-denominator column: constant WS in every head block (the
    # masked keys contribute exp(~-30) ~ 0, so no mask needed here)
    dencols = v_sb.rearrange("p t (h c) -> p t h c", c=HB)[:, :, :, 64]
    nc.vector.memset(dencols, WS)

    psA = ctx.enter_context(tc.tile_pool(name="psA", bufs=2, space="PSUM"))
    psN = ctx.enter_context(tc.tile_pool(name="psN", bufs=2, space="PSUM"))
    att = ctx.enter_context(tc.tile_pool(name="att", bufs=4))
    sm = ctx.enter_context(tc.tile_pool(name="sm", bufs=2))
    dr = ctx.enter_context(tc.tile_pool(name="dr", bufs=2, space="DRAM"))

    # ---- v projection (token-major per-head blocks + bq·k column) --------
    def vproj(t, evict_act=False):
        ps = psA.tile([P, 1024], F32, tag="ps")
        for c in range(2):
            # matmul output must fit one PSUM bank -> split at 512
            for lo, hi in ((0, 512), (512, VWP)):
                nc.tensor.matmul(
                    ps[:, lo:hi],
                    vt_d[:, c, :, t * P:(t + 1) * P],
                    wvp_d[:, c, :, lo:hi],
                    start=(c == 0), stop=(c == 1),
                    perf_mode=DR,
                )
        # pure strided copy of the 64 v columns per head (WS stays in);
        # early chunks evict on ACT (idle until the first exp), later ones
        # on DVE so they never delay the exp stream
        src = ps[:, 0:VW].rearrange("p (h c) -> p h c", c=HB)[:, :, 0:64]
        dst = v_sb[:, t, :].rearrange("p (h c) -> p h c", c=HB)[:, :, 0:64]
        nc.vector.tensor_copy(dst, src)
        # per-head exp bias: bneg + bq_h·k_h·SCALE/WS  (col 65 of each block)
        bqk = ps[:, 0:VW].rearrange("p (h c) -> p h c", c=HB)[:, :, 65]
        nc.vector.tensor_scalar(
            ebias_sb[:, t, :], bqk, SCALE / WS,
            bneg_sb[:, t:t + 1], op0=OP.mult, op1=OP.add,
        )

    def kproj(fc, evict_act=False, pool=None):
        for kh in range(2):
            ps = (pool or psA).tile([P, 1024], F32, tag="num" if pool else "ps")
            for c in range(2):
                for j in range(2):
                    ksl = slice(kh * 1024 + j * 512, kh * 1024 + (j + 1) * 512)
                    nc.tensor.matmul(
                        ps[:, j * 512:(j + 1) * 512],
                        wk_d[:, c, :, fc * P:(fc + 1) * P],
                        vt_d[:, c, :, ksl],
                        start=(c == 0), stop=(c == 1),
                        perf_mode=DR,
                    )
            dst = kt_sb[:, fc, kh * 1024:(kh + 1) * 1024]
            if evict_act:
                nc.scalar.activation(dst, ps, AF.Copy)
            else:
                nc.vector.tensor_copy(dst, ps)

    def qproj(fc, pool=None):
        ps = (pool or psA).tile([P, 1024], F32, tag="num" if pool else "ps")
        for kd in range(KD):
            for j in range(2):
                nc.tensor.matmul(
                    ps[:, j * 512:(j + 1) * 512],
                    wq_sb[:, kd, fc * P:(fc + 1) * P],
                    qtin[:, kd, j * 512:(j + 1) * 512],
                    start=(kd == 0), stop=(kd == KD - 1),
                )
        nc.vector.tensor_copy(qt_sb[:, fc, :], ps)

    def attention(h):
        fc, row = h // 2, 64 * (h % 2)
        rsl = slice(row, row + 64)
        num = psN.tile([65, 1024], F32, tag="num")
        for kc in range(TCH):
            sps = psA.tile([P, 1024], F32, tag="ps")
            for j in range(2):
                nc.tensor.matmul(
                    sps[:, j * 512:(j + 1) * 512],
                    kt_sb[rsl, fc, kc * P:(kc + 1) * P],
                    qt_sb[rsl, fc, j * 512:(j + 1) * 512],
                    start=True, stop=True,
                    tile_position=(row, 0),
                )
            es = att.tile([P, 1024], BF16, tag="es")
            nc.scalar.activation(
                es, sps, AF.Exp, bias=ebias_sb[:, kc, h:h + 1], scale=SCALE / WS
            )
            for j in range(2):
                nc.tensor.matmul(
                    num[:, j * 512:(j + 1) * 512],
                    v_sb[:, kc, HB * h:HB * h + 65],
                    es[:, j * 512:(j + 1) * 512],
                    start=(kc == 0), stop=(kc == TCH - 1),
                )
        rec = sm.tile([65, 1024], F32, tag="rec")
        nc.vector.reciprocal(rec[64:65, :], num[64:65, :])
        drec = dr.tile([1, 1024], F32, tag="drec")
        nc.sync.dma_start(drec, rec[64:65, :])
        bca = sm.tile([64, 1024], F32, tag="bca")
        nc.sync.dma_start(bca, drec.to_broadcast([64, 1024]))
        # t1 written at the head's own partition range so the fused add's
        # SBUF operands share a base partition (NCC_IBIR297)
        t1 = sm.tile([P, 1024], F32, tag="t1")
        nc.vector.tensor_tensor(t1[rsl, :], num[0:64, :], bca, op=OP.mult)
        nc.vector.scalar_tensor_tensor(
            ot_sb[rsl, fc, :], t1[rsl, :], bvq_sb[rsl, fc:fc + 1],
            qt_sb[rsl, fc, :], op0=OP.add, op1=OP.add,
        )

    # ---- schedule: h0's deps first, then per-head with trailing projs.
    # Trailing projections draw PSUM from the num pool so they never stall
    # the S/exp slot stream mid-attention.
    for t in range(TCH):
        vproj(t, evict_act=(t < 8))
    qproj(0)
    kproj(0, evict_act=True)
    for h in range(H):
        attention(h)
        if h in (0, 1, 2):
            kproj(h + 1, pool=psN)
            qproj(h + 1, pool=psN)

    # ---- output projection ----------------------------------------------
    out_dst = io["outT"].rearrange("(fc p) q -> p fc q", p=P)
    for ofc in range(FCH):
        ups = psA.tile([P, 1024], F32, tag="ps")
        for ifc in range(FCH):
            for j in range(2):
                nc.tensor.matmul(
                    ups[:, j * 512:(j + 1) * 512],
                    wo_sb[:, ifc, ofc * P:(ofc + 1) * P],
                    ot_sb[:, ifc, j * 512:(j + 1) * 512],
                    start=(ifc == 0), stop=(ifc == FCH - 1),
                )
        # relu(ups + bo) on ACT (idle after the last exp); final add on DVE
        r1 = sm.tile([P, 1024], F32, tag="r1")
        nc.scalar.activation(r1, ups, AF.Relu, bias=bo_sb[:, ofc:ofc + 1])
        fin = sm.tile([P, 1024], F32, tag="fin")
        nc.vector.tensor_tensor(fin, r1, ot_sb[:, ofc, :], op=OP.add)
        nc.sync.dma_start(out_dst[:, ofc, :], fin)


def make_core_inputs(Q, V, mask, Wq, bq, Wk, bk, Wv, bv, Wo, bo, core):
    import ml_dtypes

    f32 = np.float32
    f8 = ml_dtypes.float8_e4m3
    bf16 = ml_dtypes.bfloat16
    b, s = divmod(core, 2)

    # VTd[k, c, i, t] = V[b, t, g],  g = 256c + 128i + k
    VT = np.asarray(V[b], dtype=f32).T  # [512, 2048]
    VTd = np.ascontiguousarray(
        VT.reshape(2, 2, P, NK).transpose(2, 0, 1, 3).astype(f8)
    )
    # v-projection weights: per-head blocks of Wv^T*WS; col 528 = (bq@Wk)*WS
    WvT = np.asarray(Wv, dtype=f32).T * WS  # [512 g, 512 o]
    WVP = np.zeros((DIM, VWP), dtype=f32)
    bq32, Wk32 = np.asarray(bq, f32), np.asarray(Wk, f32)
    for h in range(H):
        WVP[:, HB * h:HB * h + 64] = WvT[:, 64 * h:64 * h + 64]
        # per-head logit shift bq_h·k_h as one extra projection column
        WVP[:, HB * h + 65] = (bq32[64 * h:64 * h + 64] @ Wk32[64 * h:64 * h + 64, :]) * WS
    WVPd = np.ascontiguousarray(
        WVP.reshape(2, 2, P, VWP).transpose(2, 0, 1, 3).astype(f8)
    )
    # WKd[k, c, i, m] = Wk[m, g]*WS  (bk dropped: shift-invariant in softmax)
    WkT = np.asarray(Wk, dtype=f32).T * WS
    WKd = np.ascontiguousarray(
        WkT.reshape(2, 2, P, DIM).transpose(2, 0, 1, 3).astype(f8)
    )
    # QTbf[k, kd, t] = Q[b, s*1024+t, 128kd+k]
    QT = np.asarray(Q[b, s * NQ:(s + 1) * NQ, :], dtype=f32).T  # [512, 1024]
    QTbf = np.ascontiguousarray(
        QT.reshape(KD, P, NQ).transpose(1, 0, 2).astype(bf16)
    )
    WqT = np.asarray(Wq, dtype=f32).T
    WQbf = np.ascontiguousarray(
        WqT.reshape(KD, P, DIM).transpose(1, 0, 2).astype(bf16)
    )
    WoT = np.asarray(Wo, dtype=f32).T
    WObf = np.ascontiguousarray(
        WoT.reshape(KD, P, DIM).transpose(1, 0, 2).astype(bf16)
    )
    # bneg[k, tc] = MB where key 128*tc+k is masked out, else 0
    m01 = np.asarray(mask[b], dtype=f32)
    bneg = np.ascontiguousarray((1.0 - m01.reshape(TCH, P).T) * MB)
    return {
        "VTd": VTd,
        "QTbf": QTbf,
        "WVPd": WVPd,
        "WKd": WKd,
        "WQbf": WQbf,
        "WObf": WObf,
        "bvq": np.asarray(bv, f32) + np.asarray(bq, f32),
        "bo": np.asarray(bo, dtype=f32),
        "bneg": bneg,
    }


_CACHE = {}


def build_program(iters=1):
    key = ("nc", iters)
    if key in _CACHE:
        return _CACHE[key]
    from contextlib import ExitStack

    nc = bacc.Bacc("TRN2", target_bir_lowering=False, debug=False)
    io = {}
    for name, (shape, dt) in INPUT_SPECS.items():
        io[name] = nc.dram_tensor(name, list(shape), dt, kind="ExternalInput").ap()
    io["outT"] = nc.dram_tensor("outT", [DIM, NQ], F32, kind="ExternalOutput").ap()
    with tile.TileContext(nc) as tc:
        for _ in range(iters):
            with ExitStack() as ctx:
                emit(ctx, tc, io)
    nc.compile()
    _CACHE[key] = nc
    return nc


def kernel(Q, V, mask, Wq, bq, Wk, bk, Wv, bv, Wo, bo):
    from concourse.bass_utils import run_bass_kernel_spmd

    nc = build_program()
    args = (Q, V, mask, Wq, bq, Wk, bk, Wv, bv, Wo, bo)
    in_maps = [make_core_inputs(*args, core=c) for c in range(8)]
    res = run_bass_kernel_spmd(
        nc, in_maps, core_ids=list(range(8)),
        trace=bool(int(os.environ.get("KTRACE", "0"))),
    )
    _CACHE["last_result"] = res
    B = 4
    out = np.empty((B, 2 * NQ, DIM), np.float32)
    for c in range(8):
        b, s = divmod(c, 2)
        out[b, s * NQ:(s + 1) * NQ, :] = res.results[c]["outT"].T
    return out


# revision 29
# speedup vs baseline: 3.6260x; 1.3194x over previous
"""MAB (multihead attention block) Trainium2 kernel, v2.

Sharding: 8 cores = 4 batches x 2 query-halves. Each core computes, for its
batch b and query half s (1024 queries), the full 8-head attention block:
    q = Q @ Wq.T + bq ; k = V @ Wk.T + bk ; v = V @ Wv.T + bv
    S = q k^T / sqrt(512); masked softmax over keys; O = q + A @ v
    out = O + relu(O @ Wo.T + bo)

Key algebraic folds (all exact up to fp rounding):
  - bk is dropped: softmax over keys is invariant to the per-query shift
    q·bk, and masked keys are excluded exactly anyway.
  - the key mask enters as a per-key exp bias (-30 => es ~ 5e-14 ~ 0)
    instead of zeroing v rows.
  - bq enters the logits as the per-key shift bq·k = (bq @ Wk) @ V — one
    extra column in the v-projection weights, added to the exp bias; the
    residual gets bq via bvq = bv + bq in the final fused add.
  - the host scales Wk/Wv by WS=64 (keeps fp8 in normal range); WS cancels:
    exp scale becomes SCALE/WS, and the softmax denominator column is WS so
    num/den is unscaled.
  With that, every PSUM evict is a pure dtype-converting copy.

Engine/dtype plan:
  - k/v projections: fp8e4 DoubleRow matmuls (contraction 2x256) with
    host-prepped [K,2,*] layouts; q/O projections and S/A@v: bf16.
  - attention per head over all 1024 queries: S^T [128 keys, 1024 q] in
    PSUM -> exp on ACT (the binding engine, ~1us per key-chunk) -> numerator
    matmul with a WS column appended to v so the masked-softmax denominator
    accumulates in the same PSUM tile.
  - normalize: reciprocal_approx_fast (DVE) + partition_broadcast (GPSIMD).
  - PSUM: two pools x 2 bufs x 2 banks = 8 banks, uniform [*, 1024] tiles.
  - DMAs chunked/ordered so the first exp fires ~12us in; all evictions run
    on DVE which trickles behind the ACT exp stream.
"""

import math
import os

import numpy as np

import concourse.bass as bass
import concourse.tile as tile
from concourse import bacc, mybir

F32 = mybir.dt.float32
BF16 = mybir.dt.bfloat16
F8 = mybir.dt.float8e4

DIM = 512
NQ = 1024  # queries per core
NK = 2048  # keys per core
H = 8
D = 64
P = 128
FCH = DIM // P  # 4 feature chunks
KD = DIM // P  # 4 contraction chunks (bf16 path)
TCH = NK // P  # 16 token/key chunks
SCALE = 1.0 / math.sqrt(DIM)
WS = 64.0  # host-side fp8 weight scale; cancels via exp-scale + den column
MB = -30.0  # masked-key exp bias: exp(s*SCALE + MB) ~ 5e-14

# v-projection output columns: 8 head blocks of [v(64) | den-slot | pad],
# then one bq·k column + pad to a 16-multiple (DoubleRow step constraint).
HB = 66
VW = H * HB  # 528: per head [v(64) | den-slot(WS) | bq_h·k_h col]
VWP = VW

INPUT_SPECS = {
    "VTd": ([P, 2, 2, NK], F8),
    "QTbf": ([P, KD, NQ], BF16),
    "WVPd": ([P, 2, 2, VWP], F8),
    "WKd": ([P, 2, 2, DIM], F8),
    "WQbf": ([P, KD, DIM], BF16),
    "WObf": ([P, KD, DIM], BF16),
    "bvq": ([DIM], F32),
    "bo": ([DIM], F32),
    "bneg": ([P, TCH], F32),
}


def emit(ctx, tc, io):
    """Emit the kernel. io: dict name -> DRAM AP (inputs + 'outT')."""
    nc = tc.nc
    AF = mybir.ActivationFunctionType
    OP = mybir.AluOpType
    DR = mybir.MatmulPerfMode.DoubleRow

    consts = ctx.enter_context(tc.tile_pool(name="consts", bufs=1))
    bigs = ctx.enter_context(tc.tile_pool(name="bigs", bufs=1))

    # ---- DMA issue order: small consts, v-proj deps, attention-h0 critical
    # path (wk, wq, qtin), then the rest ----------------------------------
    bvq_sb = consts.tile([P, FCH], F32)
    nc.sync.dma_start(bvq_sb, io["bvq"].rearrange("(c p) -> p c", p=P))
    bo_sb = consts.tile([P, FCH], F32)
    nc.sync.dma_start(bo_sb, io["bo"].rearrange("(c p) -> p c", p=P))
    bneg_sb = consts.tile([P, TCH], F32)
    nc.sync.dma_start(bneg_sb, io["bneg"])

    # warm the ACT exp table early so the ~2.7us table load overlaps DMA
    warm = consts.tile([1, 1], F32)
    nc.vector.memset(warm, 0.0)
    nc.scalar.activation(warm, warm, AF.Exp)

    wvp_d = consts.tile([P, 2, 2, VWP], F8)
    nc.sync.dma_start(wvp_d, io["WVPd"])
    vt_d = consts.tile([P, 2, 2, NK], F8)
    for tch in range(4):
        sl = slice(tch * 512, (tch + 1) * 512)
        nc.sync.dma_start(vt_d[:, :, :, sl], io["VTd"][:, :, :, sl])
    wq_sb = consts.tile([P, KD, DIM], BF16)
    nc.sync.dma_start(wq_sb, io["WQbf"])
    qtin = consts.tile([P, KD, NQ], BF16)
    for qch in range(2):
        sl = slice(qch * 512, (qch + 1) * 512)
        nc.sync.dma_start(qtin[:, :, sl], io["QTbf"][:, :, sl])
    wk_d = consts.tile([P, 2, 2, DIM], F8)
    nc.sync.dma_start(wk_d, io["WKd"])
    wo_sb = consts.tile([P, KD, DIM], BF16)
    nc.sync.dma_start(wo_sb, io["WObf"])

    # ---- persistent results ----------------------------------------------
    v_sb = bigs.tile([P, TCH, VW], BF16)
    kt_sb = bigs.tile([P, FCH, NK], BF16)
    qt_sb = bigs.tile([P, FCH, NQ], BF16)
    ot_sb = bigs.tile([P, FCH, NQ], BF16)
    ebias_sb = bigs.tile([P, TCH, H], F32)

    # softmax-denominator column: constant WS in every head block (the
    # masked keys contribute exp(~-30) ~ 0, so no mask needed here)
    dencols = v_sb.rearrange("p t (h c) -> p t h c", c=HB)[:, :, :, 64]
    nc.vector.memset(dencols, WS)

    psA = ctx.enter_context(tc.tile_pool(name="psA", bufs=2, space="PSUM"))
    psN = ctx.enter_context(tc.tile_pool(name="psN", bufs=2, space="PSUM"))
    att = ctx.enter_context(tc.tile_pool(name="att", bufs=4))
    sm = ctx.enter_context(tc.tile_pool(name="sm", bufs=2))
    dr = ctx.enter_context(tc.tile_pool(name="dr", bufs=2, space="DRAM"))

    # ---- v projection (token-major per-head blocks + bq·k column) --------
    def vproj(t, evict_act=False):
        ps = psA.tile([P, 1024], F32, tag="ps")
        for c in range(2):
            # matmul output must fit one PSUM bank -> split at 512
            for lo, hi in ((0, 512), (512, VWP)):
                nc.tensor.matmul(
                    ps[:, lo:hi],
                    vt_d[:, c, :, t * P:(t + 1) * P],
                    wvp_d[:, c, :, lo:hi],
                    start=(c == 0), stop=(c == 1),
                    perf_mode=DR,
                )
        # pure strided copy of the 64 v columns per head (WS stays in);
        # early chunks evict on ACT (idle until the first exp), later ones
        # on DVE so they never delay the exp stream
        src = ps[:, 0:VW].rearrange("p (h c) -> p h c", c=HB)[:, :, 0:64]
        dst = v_sb[:, t, :].rearrange("p (h c) -> p h c", c=HB)[:, :, 0:64]
        nc.vector.tensor_copy(dst, src)
        # per-head exp bias: bneg + bq_h·k_h·SCALE/WS  (col 65 of each block)
        bqk = ps[:, 0:VW].rearrange("p (h c) -> p h c", c=HB)[:, :, 65]
        nc.vector.tensor_scalar(
            ebias_sb[:, t, :], bqk, SCALE / WS,
            bneg_sb[:, t:t + 1], op0=OP.mult, op1=OP.add,
        )

    def kproj(fc, evict_act=False, pool=None):
        for kh in range(2):
            ps = (pool or psA).tile([P, 1024], F32, tag="num" if pool else "ps")
            for c in range(2):
                for j in range(2):
                    ksl = slice(kh * 1024 + j * 512, kh * 1024 + (j + 1) * 512)
                    nc.tensor.matmul(
                        ps[:, j * 512:(j + 1) * 512],
                        wk_d[:, c, :, fc * P:(fc + 1) * P],
                        vt_d[:, c, :, ksl],
                        start=(c == 0), stop=(c == 1),
                        perf_mode=DR,
                    )
            dst = kt_sb[:, fc, kh * 1024:(kh + 1) * 1024]
            if evict_act:
                nc.scalar.activation(dst, ps, AF.Copy)
            else:
                nc.vector.tensor_copy(dst, ps)

    def qproj(fc, pool=None):
        ps = (pool or psA).tile([P, 1024], F32, tag="num" if pool else "ps")
        for kd in range(KD):
            for j in range(2):
                nc.tensor.matmul(
                    ps[:, j * 512:(j + 1) * 512],
                    wq_sb[:, kd, fc * P:(fc + 1) * P],
                    qtin[:, kd, j * 512:(j + 1) * 512],
                    start=(kd == 0), stop=(kd == KD - 1),
                )
        nc.vector.tensor_copy(qt_sb[:, fc, :], ps)

    def attention(h):
        fc, row = h // 2, 64 * (h % 2)
        rsl = slice(row, row + 64)
        num = psN.tile([65, 1024], F32, tag="num")
        for kc in range(TCH):
            sps = psA.tile([P, 1024], F32, tag="ps")
            for j in range(2):
                nc.tensor.matmul(
                    sps[:, j * 512:(j + 1) * 512],
                    kt_sb[rsl, fc, kc * P:(kc + 1) * P],
                    qt_sb[rsl, fc, j * 512:(j + 1) * 512],
                    start=True, stop=True,
                    tile_position=(row, 0),
                )
            es = att.tile([P, 1024], BF16, tag="es")
            nc.scalar.activation(
                es, sps, AF.Exp, bias=ebias_sb[:, kc, h:h + 1], scale=SCALE / WS
            )
            for j in range(2):
                nc.tensor.matmul(
                    num[:, j * 512:(j + 1) * 512],
                    v_sb[:, kc, HB * h:HB * h + 65],
                    es[:, j * 512:(j + 1) * 512],
                    start=(kc == 0), stop=(kc == TCH - 1),
                )
        # denominator reciprocal: bounce the [1,1024] PSUM row through DRAM
        # to spread it over 64 partitions, so the exact (multi-pass) DVE
        # reciprocal runs on 16 elems/lane instead of 1024 on one lane
        den = sm.tile([65, 1024], F32, tag="rec")
        nc.vector.tensor_copy(den[64:65, :], num[64:65, :])
        dden = dr.tile([1, 1024], F32, tag="dden")
        nc.sync.dma_start(dden, den[64:65, :])
        den64 = sm.tile([64, 16], F32, tag="den64")
        nc.sync.dma_start(den64, dden.rearrange("o (c p) -> (o p) c", p=64))
        rec64 = sm.tile([64, 16], F32, tag="rec64")
        nc.vector.reciprocal(rec64, den64)
        drec = dr.tile([1, 1024], F32, tag="drec")
        nc.sync.dma_start(drec.rearrange("o (c p) -> (o p) c", p=64), rec64)
        bca = sm.tile([64, 1024], F32, tag="bca")
        nc.sync.dma_start(bca, drec.to_broadcast([64, 1024]))
        # t1 written at the head's own partition range so the fused add's
        # SBUF operands share a base partition (NCC_IBIR297)
        t1 = sm.tile([P, 1024], F32, tag="t1")
        nc.vector.tensor_tensor(t1[rsl, :], num[0:64, :], bca, op=OP.mult)
        nc.vector.scalar_tensor_tensor(
            ot_sb[rsl, fc, :], t1[rsl, :], bvq_sb[rsl, fc:fc + 1],
            qt_sb[rsl, fc, :], op0=OP.add, op1=OP.add,
        )

    # ---- schedule: h0's deps first, then per-head with trailing projs.
    # Trailing projections draw PSUM from the num pool so they never stall
    # the S/exp slot stream mid-attention.
    for t in range(TCH):
        vproj(t, evict_act=(t < 8))
    qproj(0)
    kproj(0, evict_act=True)
    for h in range(H):
        attention(h)
        if h in (0, 1, 2):
            kproj(h + 1, pool=psN)
            qproj(h + 1, pool=psN)

    # ---- output projection ----------------------------------------------
    out_dst = io["outT"].rearrange("(fc p) q -> p fc q", p=P)
    for ofc in range(FCH):
        ups = psA.tile([P, 1024], F32, tag="ps")
        for ifc in range(FCH):
            for j in range(2):
                nc.tensor.matmul(
                    ups[:, j * 512:(j + 1) * 512],
                    wo_sb[:, ifc, ofc * P:(ofc + 1) * P],
                    ot_sb[:, ifc, j * 512:(j + 1) * 512],
                    start=(ifc == 0), stop=(ifc == FCH - 1),
                )
        # relu(ups + bo) on ACT (idle after the last exp); final add on DVE
        r1 = sm.tile([P, 1024], F32, tag="r1")
        nc.scalar.activation(r1, ups, AF.Relu, bias=bo_sb[:, ofc:ofc + 1])
        fin = sm.tile([P, 1024], F32, tag="fin")
        nc.vector.tensor_tensor(fin, r1, ot_sb[:, ofc, :], op=OP.add)
        nc.sync.dma_start(out_dst[:, ofc, :], fin)


def make_core_inputs(Q, V, mask, Wq, bq, Wk, bk, Wv, bv, Wo, bo, core):
    import ml_dtypes

    f32 = np.float32
    f8 = ml_dtypes.float8_e4m3
    bf16 = ml_dtypes.bfloat16
    b, s = divmod(core, 2)

    # VTd[k, c, i, t] = V[b, t, g],  g = 256c + 128i + k
    VT = np.asarray(V[b], dtype=f32).T  # [512, 2048]
    VTd = np.ascontiguousarray(
        VT.reshape(2, 2, P, NK).transpose(2, 0, 1, 3).astype(f8)
    )
    # v-projection weights: per-head blocks of Wv^T*WS; col 528 = (bq@Wk)*WS
    WvT = np.asarray(Wv, dtype=f32).T * WS  # [512 g, 512 o]
    WVP = np.zeros((DIM, VWP), dtype=f32)
    bq32, Wk32 = np.asarray(bq, f32), np.asarray(Wk, f32)
    for h in range(H):
        WVP[:, HB * h:HB * h + 64] = WvT[:, 64 * h:64 * h + 64]
        # per-head logit shift bq_h·k_h as one extra projection column
        WVP[:, HB * h + 65] = (bq32[64 * h:64 * h + 64] @ Wk32[64 * h:64 * h + 64, :]) * WS
    WVPd = np.ascontiguousarray(
        WVP.reshape(2, 2, P, VWP).transpose(2, 0, 1, 3).astype(f8)
    )
    # WKd[k, c, i, m] = Wk[m, g]*WS  (bk dropped: shift-invariant in softmax)
    WkT = np.asarray(Wk, dtype=f32).T * WS
    WKd = np.ascontiguousarray(
        WkT.reshape(2, 2, P, DIM).transpose(2, 0, 1, 3).astype(f8)
    )
    # QTbf[k, kd, t] = Q[b, s*1024+t, 128kd+k]
    QT = np.asarray(Q[b, s * NQ:(s + 1) * NQ, :], dtype=f32).T  # [512, 1024]
    QTbf = np.ascontiguousarray(
        QT.reshape(KD, P, NQ).transpose(1, 0, 2).astype(bf16)
    )
    WqT = np.asarray(Wq, dtype=f32).T
    WQbf = np.ascontiguousarray(
        WqT.reshape(KD, P, DIM).transpose(1, 0, 2).astype(bf16)
    )
    WoT = np.asarray(Wo, dtype=f32).T
    WObf = np.ascontiguousarray(
        WoT.reshape(KD, P, DIM).transpose(1, 0, 2).astype(bf16)
    )
    # bneg[k, tc] = MB where key 128*tc+k is masked out, else 0
    m01 = np.asarray(mask[b], dtype=f32)
    bneg = np.ascontiguousarray((1.0 - m01.reshape(TCH, P).T) * MB)
    return {
        "VTd": VTd,
        "QTbf": QTbf,
        "WVPd": WVPd,
        "WKd": WKd,
        "WQbf": WQbf,
        "WObf": WObf,
        "bvq": np.asarray(bv, f32) + np.asarray(bq, f32),
        "bo": np.asarray(bo, dtype=f32),
        "bneg": bneg,
    }


_CACHE = {}


def build_program(iters=1):
    key = ("nc", iters)
    if key in _CACHE:
        return _CACHE[key]
    from contextlib import ExitStack

    nc = bacc.Bacc("TRN2", target_bir_lowering=False, debug=False)
    io = {}
    for name, (shape, dt) in INPUT_SPECS.items():
        io[name] = nc.dram_tensor(name, list(shape), dt, kind="ExternalInput").ap()
    io["outT"] = nc.dram_tensor("outT", [DIM, NQ], F32, kind="ExternalOutput").ap()
    with tile.TileContext(nc) as tc:
        for _ in range(iters):
            with ExitStack() as ctx:
                emit(ctx, tc, io)
    nc.compile()
    _CACHE[key] = nc
    return nc


def kernel(Q, V, mask, Wq, bq, Wk, bk, Wv, bv, Wo, bo):
    from concourse.bass_utils import run_bass_kernel_spmd

    nc = build_program()
    args = (Q, V, mask, Wq, bq, Wk, bk, Wv, bv, Wo, bo)
    in_maps = [make_core_inputs(*args, core=c) for c in range(8)]
    res = run_bass_kernel_spmd(
        nc, in_maps, core_ids=list(range(8)),
        trace=bool(int(os.environ.get("KTRACE", "0"))),
    )
    _CACHE["last_result"] = res
    B = 4
    out = np.empty((B, 2 * NQ, DIM), np.float32)
    for c in range(8):
        b, s = divmod(c, 2)
        out[b, s * NQ:(s + 1) * NQ, :] = res.results[c]["outT"].T
    return out
